# revision 1
# baseline (speedup 1.0000x reference)
"""Trainium2 kernel for nn_MultiHeadAttention_78683800863086.

Sparse multi-head attention with an edge-bias MLP:
  - per-head Q/K/V projections (H=8 heads, dk=dv=16) of q [B=32, N=512, D=128]
  - logits = QK^T/sqrt(dk) + MLP(edge_matrix) bias, masked softmax, AV,
    per-head output projection summed into [B, N, E=128].

Sharding: data-parallel over batch B across the 8 NeuronCores (4 batches
per core); all per-head weights are replicated.  The per-core program is
compiled once with jax.pmap onto the 8 axon-tunneled trn2 cores.
"""

import numpy as np

B, N, D, H, DK, DV, E = 32, 512, 128, 8, 16, 16, 128
NCORES = 8
Bc = B // NCORES  # batches per core

_compiled = None


def _build():
    global _compiled
    if _compiled is not None:
        return _compiled
    import jax
    import jax.numpy as jnp

    devs = jax.devices()[:NCORES]

    def percore(q, mask, edge, Wq, Wk, Wv, Wo,
                w1, b1, w2, b2, w3, b3):
        # q [Bc,N,D]; mask [Bc,N,N] bool; edge [Bc,N,N]
        norm = 1.0 / np.sqrt(DK)
        # fold the 1/sqrt(dk) scale into the query projection weights
        Q = jnp.einsum('bni,hid->hbnd', q, Wq * norm,
                       preferred_element_type=jnp.float32)
        K = jnp.einsum('bni,hid->hbnd', q, Wk,
                       preferred_element_type=jnp.float32)
        V = jnp.einsum('bni,hid->hbnd', q, Wv,
                       preferred_element_type=jnp.float32)
        comp = jnp.einsum('hbqd,hbkd->hbqk', Q, K,
                          preferred_element_type=jnp.float32)
        # edge-bias MLP on each scalar edge weight: [Bc,N,N] -> [Bc,N,N,H]
        e = edge[..., None]
        h1 = jax.nn.relu(e * w1[0] + b1)          # [Bc,N,N,16]  (1->16 is an outer product)
        h2 = jax.nn.relu(jnp.einsum('bqkj,ji->bqki', h1, w2,
                                    preferred_element_type=jnp.float32) + b2)
        bias = jnp.einsum('bqkj,jh->bqkh', h2, w3,
                          preferred_element_type=jnp.float32) + b3
        comp = comp + jnp.transpose(bias, (3, 0, 1, 2))
        m = mask[None]
        comp = jnp.where(m, -jnp.inf, comp)
        attn = jax.nn.softmax(comp, axis=-1)
        attn = jnp.where(m, 0.0, attn)
        heads = jnp.einsum('hbqk,hbkd->hbqd', attn, V,
                           preferred_element_type=jnp.float32)
        out = jnp.einsum('hbqd,hde->bqe', heads, Wo,
                         preferred_element_type=jnp.float32)
        return out

    wargs = (None,) * 10
    _compiled = (jax, jax.pmap(percore,
                               in_axes=(0, 0, 0) + wargs,
                               devices=devs))
    return _compiled


def kernel(q, mask, edge_matrix, W_query, W_key, W_val, W_out,
           mlp_W1, mlp_b1, mlp_W2, mlp_b2, mlp_W3, mlp_b3):
    jax, fn = _build()
    qs = np.ascontiguousarray(np.asarray(q, np.float32).reshape(NCORES, Bc, N, D))
    ms = np.ascontiguousarray(np.asarray(mask).reshape(NCORES, Bc, N, N))
    es = np.ascontiguousarray(
        np.asarray(edge_matrix, np.float32).reshape(NCORES, Bc, N, N))
    out = fn(qs, ms, es,
             np.asarray(W_query, np.float32), np.asarray(W_key, np.float32),
             np.asarray(W_val, np.float32), np.asarray(W_out, np.float32),
             np.asarray(mlp_W1, np.float32), np.asarray(mlp_b1, np.float32),
             np.asarray(mlp_W2, np.float32), np.asarray(mlp_b2, np.float32),
             np.asarray(mlp_W3, np.float32), np.asarray(mlp_b3, np.float32))
    return np.asarray(out).reshape(B, N, E)
